# revision 49
# baseline (speedup 1.0000x reference)
"""Cross-attention kernel for Trainium2, 8 NeuronCores (v2).

Sharding (data + head parallel, per the problem's sharding hint):
  core c in 0..7 -> batch b = c // 4, head-pair hp = c % 4.
  Each core computes attention for its batch with 2 of the 8 heads
  (a 128-wide slice of the 512 hidden features), then the partial
  out-projection  attn_out_slice @ Wo[slice, :].  The host sums the 4
  partials per batch (the "all-reduce"); bo is added on the hp==0 core.

Changes vs the 193.7us baseline (trace-driven; measured ~130us):
  - All activation/weight streaming as large single DMAs from host-swizzled
    contiguous layouts (HWDGE issue cost is a fixed ~625ns per dma_start;
    the baseline spent 130us of Sync-engine time issuing 216 small DMAs).
    The first ctx/x pieces are split into ~0.25MB parts so the projection
    chains chase the arriving data.
  - One flat software-pipelined emission over all 64 (s, mc) attention
    steps: emission order == Tile scheduler priority, so scores are
    emitted one step ahead and the ScalarE exp stream (the pacer, ~71us
    busy) crosses n-chunk boundaries without re-priming; PV / projections /
    out-projection trail as low-priority single-matmul fillers.
  - Softmax denominator reciprocals: [1,1024] den row -> SBUF->SBUF DMA
    repartition to [128,8] -> DVE reciprocal -> SBUF->SBUF DMA back to a
    [1,1024] row -> K=1 PE matmul broadcast (ones.T @ rec_row) -> DVE
    multiply.  No DRAM bounce (the baseline's path).
  - V transposed per m-chunk with one [128,128] PE transpose (identity
    matmul) into an M=65 augmented-V stationary whose ones column
    accumulates the denominators during the PV matmul.
  - bf16 output, per n-tile DMAs; HAM warm-up dummy matmuls + exp table
    preload under the initial DMA shadow.

Device-side dataflow per core (all matmuls bf16 in / f32 PSUM out):
  qT[128, N] = Wq_sl.T @ x.T          (contraction over D=1024 in 8 chunks)
  kT[128, M] = Wk_sl.T @ ctx.T ; vT likewise (shared ctx stream)
  Vall[m 128, h, mc, 0:64] = PE-transpose of vT chunks; col 64 = ones
  per n-chunk s (512 cols), per m-chunk mc (128 rows):
     st[m 128, n 1024] = [kT_h0_mc.T @ qT_h0_s | kT_h1_mc.T @ qT_h1_s]
         (two concurrent matmuls on PE row-groups 0-63 / 64-127)
     pt = exp(st * 1/8)               (ScalarE, one op per m-chunk)
     oaug_h[65, 512] += Vall_h_mc.T @ pt_h     (PSUM accum over mc)
  row 64 of oaug = softmax denominators; OT[h*64:, s] = oaug[0:64]/denom
  out[n 128, 1024] = OT_ntile.T @ Wo_sl + bo         (per n-tile, bf16 out)
"""

import numpy as np

import concourse.bass as bass
import concourse.tile as tile
from concourse import bacc, mybir
from concourse.masks import make_identity

F32 = mybir.dt.float32
F32R = mybir.dt.float32r
BF16 = mybir.dt.bfloat16

D = 1024      # model dim (contraction for projections)
SEQ = 2048    # n == m
F = 128       # features per core (2 heads x 64)
DH = 64       # head dim
NS = SEQ // 512   # 4 n-chunks of 512
NK = D // 128     # 8 contraction chunks
NM = SEQ // 128   # 16 m-chunks of 128
VPAD = 72         # PV stationary row padded to 16B-aligned stride (bf16)
SCALE = DH ** -0.5

EXP = mybir.ActivationFunctionType.Exp


def build_nc():
    nc = bacc.Bacc("TRN2", target_bir_lowering=False, debug=False)

    # host-swizzled: [128, s, k, j] -> x.T[k*128+p, s*512+j], contiguous
    x_d = nc.dram_tensor("x_sw", [128, NS * NK * 512], BF16, kind="ExternalInput")
    c_d = nc.dram_tensor("c_sw", [128, NS * NK * 512], BF16, kind="ExternalInput")
    # [128, k*128+f] = W[k*128+p, f]  (contiguous per partition)
    wq_d = nc.dram_tensor("wq", [128, NK * 128], BF16, kind="ExternalInput")
    wk_d = nc.dram_tensor("wk", [128, NK * 128], BF16, kind="ExternalInput")
    wv_d = nc.dram_tensor("wv", [128, NK * 128], BF16, kind="ExternalInput")
    wo_d = nc.dram_tensor("wo", [F, D], BF16, kind="ExternalInput")
    bo_d = nc.dram_tensor("bo", [1, D], BF16, kind="ExternalInput")
    # [128, s*4096 + nt*1024 + d] = out[(s*4+nt)*128 + p, d]
    out_d = nc.dram_tensor("out_sw", [128, NS * 4 * 1024], BF16, kind="ExternalOutput")

    with tile.TileContext(nc) as tc:
        _emit(tc, nc, x_d, c_d, wq_d, wk_d, wv_d, wo_d, bo_d, out_d)
    nc.compile()
    return nc


def _emit(tc, nc, x_d, c_d, wq_d, wk_d, wv_d, wo_d, bo_d, out_d):
    from contextlib import ExitStack

    ctx = ExitStack()
    wpool = ctx.enter_context(tc.tile_pool(name="wpool", bufs=1))
    big = ctx.enter_context(tc.tile_pool(name="big", bufs=1))
    ppool = ctx.enter_context(tc.tile_pool(name="ppool", bufs=16))
    fpool = ctx.enter_context(tc.tile_pool(name="fpool", bufs=2))
    ps_st = ctx.enter_context(tc.tile_pool(name="ps_st", bufs=2, space="PSUM"))
    ps_oaug = ctx.enter_context(tc.tile_pool(name="ps_oaug", bufs=2, space="PSUM"))
    ps_acc = ctx.enter_context(tc.tile_pool(name="ps_acc", bufs=2, space="PSUM"))

    # ---- SBUF tiles ----
    xs = big.tile([128, NS, NK, 512], BF16, name="xs")
    cs = big.tile([128, NS, NK, 512], BF16, name="cs")
    wq_s = wpool.tile([128, NK, 128], BF16, name="wq_s")
    wk_s = wpool.tile([128, NK, 128], BF16, name="wk_s")
    wv_s = wpool.tile([128, NK, 128], BF16, name="wv_s")
    wo_s = wpool.tile([128, D], BF16, name="wo_s")
    bo_rep = wpool.tile([128, D], BF16, name="bo_rep")
    warm = wpool.tile([128, 8], F32, name="warm")
    nc.vector.memset(warm, 0.0)

    qT = big.tile([128, SEQ], BF16, name="qT")
    kT = big.tile([128, SEQ], BF16, name="kT")
    vT = big.tile([128, SEQ], BF16, name="vT")
    OT = big.tile([128, SEQ], BF16, name="OT")
    ident = wpool.tile([128, 128], BF16, name="ident")
    make_identity(nc, ident)
    # V per head+m-chunk with a ones column (65th) that accumulates the
    # softmax denominators during the PV matmul.  VPAD keeps the per-chunk
    # stride 16B-aligned for the LDWEIGHTS access pattern.
    Vall = big.tile([128, 2, NM, VPAD], BF16, name="Vall")
    ones_sb = wpool.tile([128, 2 * NM], F32, name="ones_sb")
    nc.vector.memset(ones_sb, 1.0)
    nc.vector.tensor_copy(
        out=Vall[:, :, :, DH : DH + 1],
        in_=ones_sb.rearrange("p (h m o) -> p h m o", h=2, o=1),
    )
    ones1 = wpool.tile([1, DH], BF16, name="ones1")
    nc.vector.memset(ones1, 1.0)

    # ---- streaming loads (order == issue order on the Sync queue).
    # ctx/Wk/Wv first: the k-projection chain is the head of the whole
    # attention pipeline, so its data must land first.
    def load_piece(dst, src_d, s):
        nc.sync.dma_start(
            out=dst[:, s], in_=src_d.ap()[:, s * NK * 512 : (s + 1) * NK * 512]
        )

    def load_part(dst, src_d, s, k0, k1):
        nc.sync.dma_start(
            out=dst[:, s, k0:k1],
            in_=src_d.ap()[:, (s * NK + k0) * 512 : (s * NK + k1) * 512],
        )

    def load_cols(dst, src_d, s, j0, j1):
        src = src_d.ap().rearrange("p (s k j) -> p s k j", s=NS, j=512)
        nc.sync.dma_start(out=dst[:, s, :, j0:j1], in_=src[:, s, :, j0:j1])

    nc.sync.dma_start(out=wk_s, in_=wk_d.ap())
    # preload the exp table set under the DMA shadow
    nc.scalar.activation(out=warm, in_=warm, func=EXP, bias=0.0, scale=1.0)
    load_part(cs, c_d, 0, 0, 2)
    load_part(cs, c_d, 0, 2, 4)
    nc.sync.dma_start(out=wq_s, in_=wq_d.ap())
    load_part(cs, c_d, 0, 4, 6)
    load_part(cs, c_d, 0, 6, 8)
    load_part(xs, x_d, 0, 0, 2)
    load_part(xs, x_d, 0, 2, 4)
    nc.sync.dma_start(out=wv_s, in_=wv_d.ap())
    load_part(xs, x_d, 0, 4, 6)
    load_part(xs, x_d, 0, 6, 8)
    load_part(cs, c_d, 1, 0, 2)
    load_part(cs, c_d, 1, 2, 4)
    load_part(cs, c_d, 1, 4, 6)
    load_part(cs, c_d, 1, 6, 8)
    load_part(cs, c_d, 2, 0, 2)
    load_part(cs, c_d, 2, 2, 4)
    load_part(cs, c_d, 2, 4, 6)
    load_part(cs, c_d, 2, 6, 8)
    load_part(cs, c_d, 3, 0, 2)
    load_part(cs, c_d, 3, 2, 4)
    load_part(cs, c_d, 3, 4, 6)
    load_part(cs, c_d, 3, 6, 8)
    nc.sync.dma_start(out=wo_s, in_=wo_d.ap())
    load_piece(xs, x_d, 1)
    load_piece(xs, x_d, 2)
    load_piece(xs, x_d, 3)
    nc.gpsimd.dma_start(out=bo_rep, in_=bo_d.ap()[0, :].partition_broadcast(128))

    # HAM warm-up: dummy matmuls keep the PE clock at 8/8 while the first
    # data DMAs land, so the projection chains run at full rate.
    dummy = wpool.tile([128, 512], BF16, name="dummy")
    nc.vector.memset(dummy, 0.0)
    for _ in range(8):
        wst = ps_st.tile([128, 1024], F32, name="st", tag="st")
        nc.tensor.matmul(wst[:, 0:512], ident, dummy, start=True, stop=True)

    # ---- helpers ----
    def qproj(s):
        acc = ps_acc.tile([128, 512], F32, name="acc", tag="acc")
        for k in range(NK):
            nc.tensor.matmul(
                acc, wq_s[:, k, :], xs[:, s, k, :],
                start=(k == 0), stop=(k == NK - 1),
            )
        nc.vector.tensor_copy(out=qT[:, s * 512 : (s + 1) * 512], in_=acc)

    def k_proj_cols(g, j0, j1):
        kacc = ps_acc.tile([128, 512], F32, name="kacc", tag="acc")
        for k in range(NK):
            nc.tensor.matmul(
                kacc[:, 0 : j1 - j0], wk_s[:, k, :], cs[:, g, k, j0:j1],
                start=(k == 0), stop=(k == NK - 1),
            )
        nc.vector.tensor_copy(
            out=kT[:, g * 512 + j0 : g * 512 + j1], in_=kacc[:, 0 : j1 - j0]
        )

    def k_proj(g):
        kacc = ps_acc.tile([128, 512], F32, name="kacc", tag="acc")
        for k in range(NK):
            nc.tensor.matmul(
                kacc, wk_s[:, k, :], cs[:, g, k, :],
                start=(k == 0), stop=(k == NK - 1),
            )
        nc.vector.tensor_copy(out=kT[:, g * 512 : (g + 1) * 512], in_=kacc)

    def v_proj(g):
        vacc = ps_acc.tile([128, 512], F32, name="vacc", tag="acc")
        for k in range(NK):
            nc.tensor.matmul(
                vacc, wv_s[:, k, :], cs[:, g, k, :],
                start=(k == 0), stop=(k == NK - 1),
            )
        nc.vector.tensor_copy(out=vT[:, g * 512 : (g + 1) * 512], in_=vacc)

    def v_trans(g):
        for mc in range(4 * g, 4 * g + 4):
            tp = ps_acc.tile([128, 128], BF16, name="tp", tag="acc")
            nc.tensor.transpose(tp, vT[:, mc * 128 : (mc + 1) * 128], ident)
            nc.vector.tensor_copy(
                out=Vall[:, :, mc, 0:DH],
                in_=tp.rearrange("p (h d) -> p h d", h=2),
            )

    def scores_exp(s, mc):
        n0, n1 = s * 512, (s + 1) * 512
        m0, m1 = mc * 128, (mc + 1) * 128
        st = ps_st.tile([128, 1024], F32, name="st", tag="st")
        nc.tensor.matmul(
            st[:, 0:512], kT[0:DH, m0:m1], qT[0:DH, n0:n1],
            start=True, stop=True, tile_position=(0, 0),
        )
        nc.tensor.matmul(
            st[:, 512:1024], kT[DH:128, m0:m1], qT[DH:128, n0:n1],
            start=True, stop=True, tile_position=(64, 0),
        )
        pt = ppool.tile([128, 1024], BF16, name="pt", tag="pt")
        nc.scalar.activation(out=pt, in_=st, func=EXP, bias=0.0, scale=SCALE)
        return pt

    def pv(mc, pt, oaug):
        first, last = mc == 0, mc == NM - 1
        for h in range(2):
            nc.tensor.matmul(
                oaug[h], Vall[:, h, mc, 0 : DH + 1],
                pt[:, h * 512 : (h + 1) * 512],
                start=first, stop=last,
            )

    def attn_mc(s, mc, oaug):
        pv(mc, scores_exp(s, mc), oaug)

    def mk_oaug():
        return [
            ps_oaug.tile([DH + 1, 512], F32, name=f"oaug{h}", tag="oaug")
            for h in range(2)
        ]

    def fin_pre(s, oaug, last=False):
        """Evacuate oaug, extract denominators, compute reciprocals.

        The [1, 1024] denominator row (row 64 of oaug) is repartitioned to
        [128, 8] via an SBUF->SBUF DMA so the iterative-divide reciprocal
        runs on all DVE lanes, then linearized back to a single-partition
        [1, 1024] row for the PE broadcast in fin_post.
        """
        oaug_sb = fpool.tile([DH + 1, 1024], F32, name="oaug_sb", tag="oaug_sb")
        if last:
            # tail: pull the den row via the (idle) ScalarE concurrently
            # with the DVE evacuation so the recip chain starts sooner
            den_sb = fpool.tile([1, 1024], F32, name="den_sb", tag="den_sb")
            for h in range(2):
                nc.scalar.copy(
                    out=den_sb[:, h * 512 : (h + 1) * 512],
                    in_=oaug[h][DH : DH + 1, :],
                )
                nc.vector.tensor_copy(
                    out=oaug_sb[0:DH, h * 512 : (h + 1) * 512],
                    in_=oaug[h][0:DH, :],
                )
            den_src = den_sb
        else:
            for h in range(2):
                nc.vector.tensor_copy(
                    out=oaug_sb[:, h * 512 : (h + 1) * 512], in_=oaug[h]
                )
            den_src = oaug_sb[DH : DH + 1, :]
        den_p = fpool.tile([128, 8], F32, name="den_p", tag="den_p")
        nc.sync.dma_start(out=den_p, in_=den_src)
        rec_p = fpool.tile([128, 8], BF16, name="rec_p", tag="rec_p")
        with nc.allow_low_precision(reason="softmax denom reciprocal at bf16"):
            nc.vector.reciprocal(out=rec_p, in_=den_p)
        rec_row = fpool.tile([1, 1024], BF16, name="rec_row", tag="rec_row")
        nc.sync.dma_start(out=rec_row, in_=rec_p)
        return oaug_sb, rec_row

    def fin_post(s, oaug_sb, rec_row):
        """OT[:, s-slice] = oaug[0:64] * (1/den), denominators broadcast
        across partitions with a K=1 PE matmul (rep = ones.T @ rec_row)."""
        n0, n1 = s * 512, (s + 1) * 512
        for h in range(2):
            rep = ps_acc.tile([DH, 512], F32, name="rep", tag="acc")
            nc.tensor.matmul(
                rep, ones1, rec_row[:, h * 512 : (h + 1) * 512],
                start=True, stop=True,
            )
            nc.vector.tensor_mul(
                out=OT[h * DH : (h + 1) * DH, n0:n1],
                in0=oaug_sb[0:DH, h * 512 : (h + 1) * 512],
                in1=rep,
            )

    def mk_osb():
        return fpool.tile([128, 4, 1024], BF16, name="osb", tag="osb")

    def outproj_nt(s, nt, osb):
        col = (s * 4 + nt) * 128
        for piece in range(2):
            c0, c1 = piece * 512, (piece + 1) * 512
            ops = ps_acc.tile([128, 512], F32, name="ops", tag="acc")
            nc.tensor.matmul(
                ops, OT[:, col : col + 128], wo_s[:, c0:c1],
                start=True, stop=True,
            )
            nc.vector.tensor_add(
                out=osb[:, nt, c0:c1], in0=ops, in1=bo_rep[:, c0:c1]
            )
        nc.sync.dma_start(
            out=out_d.ap()[:, (s * 4 + nt) * 1024 : (s * 4 + nt + 1) * 1024],
            in_=osb[:, nt, :],
        )

    qaccs = {}

    def qproj_part(s, k):
        """One k-chunk of the q projection (split so the scheduler can slot
        single matmuls into the ScalarE-paced slack without stalling the
        scores chain)."""
        if k == 0:
            qaccs[s] = ps_acc.tile([128, 512], F32, name="acc", tag="acc")
        acc = qaccs[s]
        nc.tensor.matmul(
            acc, wq_s[:, k, :], xs[:, s, k, :],
            start=(k == 0), stop=(k == NK - 1),
        )
        if k == NK - 1:
            nc.vector.tensor_copy(out=qT[:, s * 512 : (s + 1) * 512], in_=acc)

    def outproj_piece(s, nt, piece, osb):
        col = (s * 4 + nt) * 128
        c0, c1 = piece * 512, (piece + 1) * 512
        ops = ps_acc.tile([128, 512], F32, name="ops", tag="acc")
        nc.tensor.matmul(
            ops, OT[:, col : col + 128], wo_s[:, c0:c1], start=True, stop=True
        )
        nc.vector.tensor_add(out=osb[:, nt, c0:c1], in0=ops, in1=bo_rep[:, c0:c1])
        if piece == 1:
            nc.sync.dma_start(
                out=out_d.ap()[:, (s * 4 + nt) * 1024 : (s * 4 + nt + 1) * 1024],
                in_=osb[:, nt, :],
            )

    # ---- schedule ----
    # Emission order == scheduler priority.  One flat software-pipelined
    # loop over all 64 (s, mc) attention steps: the scores for step i+1 are
    # emitted before the PV of step i, so the ScalarE exp stream (the
    # kernel's pacer, ~71us) is one continuous priority band that crosses
    # n-chunk boundaries; projections / out-projection / v-transposes are
    # emitted after (lower priority) and fill the PE slack.
    oaug_cur = mk_oaug()
    osb_cur = None
    k_proj(0)
    qproj(0)
    pts = {0: scores_exp(0, 0)}
    pts[1] = scores_exp(0, 1)
    for i in range(NS * NM):
        s, mc = divmod(i, NM)
        ni = i + 2
        if ni < NS * NM:
            ns_, nmc = divmod(ni, NM)
            if ns_ == 0 and nmc % 4 == 0:
                k_proj(nmc // 4)
            pts[ni] = scores_exp(ns_, nmc)
        if s == 0:
            # v/transpose groups trail the k-chain by data arrival; PV lags
            # 3 steps so its slot grants come after the next k-projection's.
            if mc % 4 == 3 and mc < 15:
                g = mc // 4
                v_proj(g)
                v_trans(g)
            if mc == 8:
                qproj(1)
            if mc >= 3 and mc < 15:
                pv(mc - 3, pts.pop(i - 3), oaug_cur)
            if mc == 15:
                v_proj(3)
                v_trans(3)
                for j in range(12, 16):
                    pv(j, pts.pop(s * NM + j), oaug_cur)
        else:
            pv(mc, pts.pop(i), oaug_cur)
            if 4 <= mc <= 11 and s + 1 < NS:
                qproj_part(s + 1, mc - 4)
            if 6 <= mc <= 13:
                outproj_piece(s - 1, (mc - 6) // 2, (mc - 6) % 2, osb_cur)
        if s >= 1 and mc == 1:
            fin_post(s - 1, *fin_pending)
        if mc == NM - 1:
            fin_pending = fin_pre(s, oaug_cur, last=(s == NS - 1))
            if s + 1 < NS:
                oaug_cur = mk_oaug()
                osb_cur = mk_osb()

    fin_post(NS - 1, *fin_pending)
    osb_cur = mk_osb()
    for nt in range(4):
        for piece in range(2):
            outproj_piece(NS - 1, nt, piece, osb_cur)

    ctx.close()


_NC = None


def _get_nc():
    global _NC
    if _NC is None:
        _NC = build_nc()
    return _NC


def _bf16():
    import ml_dtypes

    return ml_dtypes.bfloat16


def _swizzle_w(w):
    """[1024, 128] -> [128, 8*128]: chunk k of the contraction dim lands in
    column block k, so the device DMA is fully contiguous."""
    return np.ascontiguousarray(
        np.asarray(w, np.float32).reshape(NK, 128, F).transpose(1, 0, 2)
        .reshape(128, NK * F).astype(_bf16())
    )


def _swizzle_act(a):
    """[n=2048, d=1024] -> [128, (s, k, j)] with [p, s*4096 + k*512 + j] =
    a[s*512 + j, k*128 + p]."""
    at = np.asarray(a, np.float32).T  # [1024, 2048]
    return np.ascontiguousarray(
        at.reshape(NK, 128, NS, 512).transpose(1, 2, 0, 3).reshape(128, NS * NK * 512)
        .astype(_bf16())
    )


def shard_inputs(x, context, Wq, Wk, Wv, Wo, bo):
    x = np.asarray(x, np.float32)
    context = np.asarray(context, np.float32)
    Wq = np.asarray(Wq, np.float32)
    Wk = np.asarray(Wk, np.float32)
    Wv = np.asarray(Wv, np.float32)
    Wo = np.asarray(Wo, np.float32)
    bo = np.asarray(bo, np.float32)

    bf = _bf16()
    x_sw = [_swizzle_act(x[b]) for b in range(x.shape[0])]
    c_sw = [_swizzle_act(context[b]) for b in range(context.shape[0])]
    zero_bo = np.zeros((1, D), bf)
    in_maps = []
    for c in range(8):
        b, hp = divmod(c, 4)
        f0 = hp * F
        in_maps.append(
            {
                "x_sw": x_sw[b],
                "c_sw": c_sw[b],
                "wq": _swizzle_w(Wq[:, f0 : f0 + F]),
                "wk": _swizzle_w(Wk[:, f0 : f0 + F]),
                "wv": _swizzle_w(Wv[:, f0 : f0 + F]),
                "wo": np.ascontiguousarray(Wo[f0 : f0 + F, :]).astype(bf),
                "bo": bo.reshape(1, D).astype(bf) if hp == 0 else zero_bo,
            }
        )
    return in_maps


def unswizzle_out(res):
    """[128, NS*4*1024] bf16 -> [2048, 1024] f32."""
    r = np.asarray(res, np.float32).reshape(128, NS, 4, 1024)
    return r.transpose(1, 2, 0, 3).reshape(SEQ, D)


def kernel(x, context, Wq, Wk, Wv, Wo, bo):
    from concourse.bass_utils import run_bass_kernel_spmd

    in_maps = shard_inputs(x, context, Wq, Wk, Wv, Wo, bo)
    nc = _get_nc()
    res = run_bass_kernel_spmd(nc, in_maps, list(range(8)))
    out = np.zeros((2, SEQ, D), np.float32)
    for c in range(8):
        out[c // 4] += unswizzle_out(res.results[c]["out_sw"])
    return out


# revision 50
# speedup vs baseline: 1.1749x; 1.1749x over previous
"""Cross-attention kernel for Trainium2, 8 NeuronCores (v2).

Sharding (data + head parallel, per the problem's sharding hint):
  core c in 0..7 -> batch b = c // 4, head-pair hp = c % 4.
  Each core computes attention for its batch with 2 of the 8 heads
  (a 128-wide slice of the 512 hidden features), then the partial
  out-projection  attn_out_slice @ Wo[slice, :].  The host sums the 4
  partials per batch (the "all-reduce"); bo is added on the hp==0 core.

Changes vs the 193.7us baseline (trace-driven; measured ~130us):
  - All activation/weight streaming as large single DMAs from host-swizzled
    contiguous layouts (HWDGE issue cost is a fixed ~625ns per dma_start;
    the baseline spent 130us of Sync-engine time issuing 216 small DMAs).
    The first ctx/x pieces are split into ~0.25MB parts so the projection
    chains chase the arriving data.
  - One flat software-pipelined emission over all 64 (s, mc) attention
    steps: emission order == Tile scheduler priority, so scores are
    emitted one step ahead and the ScalarE exp stream (the pacer, ~71us
    busy) crosses n-chunk boundaries without re-priming; PV / projections /
    out-projection trail as low-priority single-matmul fillers.
  - Softmax denominator reciprocals: [1,1024] den row -> SBUF->SBUF DMA
    repartition to [128,8] -> DVE reciprocal -> SBUF->SBUF DMA back to a
    [1,1024] row -> K=1 PE matmul broadcast (ones.T @ rec_row) -> DVE
    multiply.  No DRAM bounce (the baseline's path).
  - V transposed per m-chunk with one [128,128] PE transpose (identity
    matmul) into an M=65 augmented-V stationary whose ones column
    accumulates the denominators during the PV matmul.
  - bf16 output, per n-tile DMAs; HAM warm-up dummy matmuls + exp table
    preload under the initial DMA shadow.

Device-side dataflow per core (all matmuls bf16 in / f32 PSUM out):
  qT[128, N] = Wq_sl.T @ x.T          (contraction over D=1024 in 8 chunks)
  kT[128, M] = Wk_sl.T @ ctx.T ; vT likewise (shared ctx stream)
  Vall[m 128, h, mc, 0:64] = PE-transpose of vT chunks; col 64 = ones
  per n-chunk s (512 cols), per m-chunk mc (128 rows):
     st[m 128, n 1024] = [kT_h0_mc.T @ qT_h0_s | kT_h1_mc.T @ qT_h1_s]
         (two concurrent matmuls on PE row-groups 0-63 / 64-127)
     pt = exp(st * 1/8)               (ScalarE, one op per m-chunk)
     oaug_h[65, 512] += Vall_h_mc.T @ pt_h     (PSUM accum over mc)
  row 64 of oaug = softmax denominators; OT[h*64:, s] = oaug[0:64]/denom
  out[n 128, 1024] = OT_ntile.T @ Wo_sl + bo         (per n-tile, bf16 out)
"""

import numpy as np

import concourse.bass as bass
import concourse.tile as tile
from concourse import bacc, mybir
from concourse.masks import make_identity

F32 = mybir.dt.float32
F32R = mybir.dt.float32r
BF16 = mybir.dt.bfloat16

D = 1024      # model dim (contraction for projections)
SEQ = 2048    # n == m
F = 128       # features per core (2 heads x 64)
DH = 64       # head dim
NS = SEQ // 512   # 4 n-chunks of 512
NK = D // 128     # 8 contraction chunks
NM = SEQ // 128   # 16 m-chunks of 128
VPAD = 72         # PV stationary row padded to 16B-aligned stride (bf16)
SCALE = DH ** -0.5

EXP = mybir.ActivationFunctionType.Exp


def build_nc():
    nc = bacc.Bacc("TRN2", target_bir_lowering=False, debug=False)

    # host-swizzled: [128, s, k, j] -> x.T[k*128+p, s*512+j], contiguous
    x_d = nc.dram_tensor("x_sw", [128, NS * NK * 512], BF16, kind="ExternalInput")
    c_d = nc.dram_tensor("c_sw", [128, NS * NK * 512], BF16, kind="ExternalInput")
    # [128, k*128+f] = W[k*128+p, f]  (contiguous per partition)
    wq_d = nc.dram_tensor("wq", [128, NK * 128], BF16, kind="ExternalInput")
    wk_d = nc.dram_tensor("wk", [128, NK * 128], BF16, kind="ExternalInput")
    wv_d = nc.dram_tensor("wv", [128, NK * 128], BF16, kind="ExternalInput")
    wo_d = nc.dram_tensor("wo", [F, D], BF16, kind="ExternalInput")
    bo_d = nc.dram_tensor("bo", [1, D], BF16, kind="ExternalInput")
    # [128, s*4096 + nt*1024 + d] = out[(s*4+nt)*128 + p, d]
    out_d = nc.dram_tensor("out_sw", [128, NS * 4 * 1024], BF16, kind="ExternalOutput")

    with tile.TileContext(nc) as tc:
        _emit(tc, nc, x_d, c_d, wq_d, wk_d, wv_d, wo_d, bo_d, out_d)
    nc.compile()
    return nc


def _emit(tc, nc, x_d, c_d, wq_d, wk_d, wv_d, wo_d, bo_d, out_d):
    from contextlib import ExitStack

    ctx = ExitStack()
    wpool = ctx.enter_context(tc.tile_pool(name="wpool", bufs=1))
    big = ctx.enter_context(tc.tile_pool(name="big", bufs=1))
    ppool = ctx.enter_context(tc.tile_pool(name="ppool", bufs=16))
    fpool = ctx.enter_context(tc.tile_pool(name="fpool", bufs=2))
    ps_st = ctx.enter_context(tc.tile_pool(name="ps_st", bufs=2, space="PSUM"))
    ps_oaug = ctx.enter_context(tc.tile_pool(name="ps_oaug", bufs=2, space="PSUM"))
    ps_acc = ctx.enter_context(tc.tile_pool(name="ps_acc", bufs=2, space="PSUM"))

    # ---- SBUF tiles ----
    xs = big.tile([128, NS, NK, 512], BF16, name="xs")
    cs = big.tile([128, NS, NK, 512], BF16, name="cs")
    wq_s = wpool.tile([128, NK, 128], BF16, name="wq_s")
    wk_s = wpool.tile([128, NK, 128], BF16, name="wk_s")
    wv_s = wpool.tile([128, NK, 128], BF16, name="wv_s")
    wo_s = wpool.tile([128, D], BF16, name="wo_s")
    bo_rep = wpool.tile([128, D], BF16, name="bo_rep")
    warm = wpool.tile([128, 8], F32, name="warm")
    nc.vector.memset(warm, 0.0)

    qT = big.tile([128, SEQ], BF16, name="qT")
    kT = big.tile([128, SEQ], BF16, name="kT")
    vT = big.tile([128, SEQ], BF16, name="vT")
    OT = big.tile([128, SEQ], BF16, name="OT")
    ident = wpool.tile([128, 128], BF16, name="ident")
    make_identity(nc, ident)
    # V per head+m-chunk with a ones column (65th) that accumulates the
    # softmax denominators during the PV matmul.  VPAD keeps the per-chunk
    # stride 16B-aligned for the LDWEIGHTS access pattern.
    Vall = big.tile([128, 2, NM, VPAD], BF16, name="Vall")
    ones_sb = wpool.tile([128, 2 * NM], F32, name="ones_sb")
    nc.vector.memset(ones_sb, 1.0)
    nc.vector.tensor_copy(
        out=Vall[:, :, :, DH : DH + 1],
        in_=ones_sb.rearrange("p (h m o) -> p h m o", h=2, o=1),
    )
    ones1 = wpool.tile([1, DH], BF16, name="ones1")
    nc.vector.memset(ones1, 1.0)

    # ---- streaming loads (order == issue order on the Sync queue).
    # ctx/Wk/Wv first: the k-projection chain is the head of the whole
    # attention pipeline, so its data must land first.
    def load_piece(dst, src_d, s):
        nc.sync.dma_start(
            out=dst[:, s], in_=src_d.ap()[:, s * NK * 512 : (s + 1) * NK * 512]
        )

    def load_part(dst, src_d, s, k0, k1):
        nc.sync.dma_start(
            out=dst[:, s, k0:k1],
            in_=src_d.ap()[:, (s * NK + k0) * 512 : (s * NK + k1) * 512],
        )

    def load_cols(dst, src_d, s, j0, j1):
        src = src_d.ap().rearrange("p (s k j) -> p s k j", s=NS, j=512)
        nc.sync.dma_start(out=dst[:, s, :, j0:j1], in_=src[:, s, :, j0:j1])

    nc.sync.dma_start(out=wk_s, in_=wk_d.ap())
    # preload the exp table set under the DMA shadow
    nc.scalar.activation(out=warm, in_=warm, func=EXP, bias=0.0, scale=1.0)
    load_part(cs, c_d, 0, 0, 2)
    load_part(cs, c_d, 0, 2, 4)
    nc.sync.dma_start(out=wq_s, in_=wq_d.ap())
    load_part(cs, c_d, 0, 4, 6)
    load_part(cs, c_d, 0, 6, 8)
    load_part(xs, x_d, 0, 0, 2)
    load_part(xs, x_d, 0, 2, 4)
    nc.sync.dma_start(out=wv_s, in_=wv_d.ap())
    load_part(xs, x_d, 0, 4, 6)
    load_part(xs, x_d, 0, 6, 8)
    load_part(cs, c_d, 1, 0, 4)
    load_part(cs, c_d, 1, 4, 8)
    load_part(cs, c_d, 2, 0, 4)
    load_part(cs, c_d, 2, 4, 8)
    load_part(cs, c_d, 3, 0, 4)
    load_part(cs, c_d, 3, 4, 8)
    nc.sync.dma_start(out=wo_s, in_=wo_d.ap())
    load_piece(xs, x_d, 1)
    load_piece(xs, x_d, 2)
    load_piece(xs, x_d, 3)
    nc.gpsimd.dma_start(out=bo_rep, in_=bo_d.ap()[0, :].partition_broadcast(128))

    # HAM warm-up: dummy matmuls keep the PE clock at 8/8 while the first
    # data DMAs land, so the projection chains run at full rate.
    dummy = wpool.tile([128, 512], BF16, name="dummy")
    nc.vector.memset(dummy, 0.0)
    for _ in range(8):
        wst = ps_st.tile([128, 1024], F32, name="st", tag="st")
        nc.tensor.matmul(wst[:, 0:512], ident, dummy, start=True, stop=True)

    # ---- helpers ----
    def qproj(s):
        acc = ps_acc.tile([128, 512], F32, name="acc", tag="acc")
        for k in range(NK):
            nc.tensor.matmul(
                acc, wq_s[:, k, :], xs[:, s, k, :],
                start=(k == 0), stop=(k == NK - 1),
            )
        nc.vector.tensor_copy(out=qT[:, s * 512 : (s + 1) * 512], in_=acc)

    def k_proj_cols(g, j0, j1):
        kacc = ps_acc.tile([128, 512], F32, name="kacc", tag="acc")
        for k in range(NK):
            nc.tensor.matmul(
                kacc[:, 0 : j1 - j0], wk_s[:, k, :], cs[:, g, k, j0:j1],
                start=(k == 0), stop=(k == NK - 1),
            )
        nc.vector.tensor_copy(
            out=kT[:, g * 512 + j0 : g * 512 + j1], in_=kacc[:, 0 : j1 - j0]
        )

    def k_proj(g):
        kacc = ps_acc.tile([128, 512], F32, name="kacc", tag="acc")
        for k in range(NK):
            nc.tensor.matmul(
                kacc, wk_s[:, k, :], cs[:, g, k, :],
                start=(k == 0), stop=(k == NK - 1),
            )
        nc.vector.tensor_copy(out=kT[:, g * 512 : (g + 1) * 512], in_=kacc)

    def v_proj(g):
        vacc = ps_acc.tile([128, 512], F32, name="vacc", tag="acc")
        for k in range(NK):
            nc.tensor.matmul(
                vacc, wv_s[:, k, :], cs[:, g, k, :],
                start=(k == 0), stop=(k == NK - 1),
            )
        nc.vector.tensor_copy(out=vT[:, g * 512 : (g + 1) * 512], in_=vacc)

    def v_trans(g):
        for mc in range(4 * g, 4 * g + 4):
            tp = ps_acc.tile([128, 128], BF16, name="tp", tag="acc")
            nc.tensor.transpose(tp, vT[:, mc * 128 : (mc + 1) * 128], ident)
            nc.vector.tensor_copy(
                out=Vall[:, :, mc, 0:DH],
                in_=tp.rearrange("p (h d) -> p h d", h=2),
            )

    def scores_exp(s, mc):
        n0, n1 = s * 512, (s + 1) * 512
        m0, m1 = mc * 128, (mc + 1) * 128
        st = ps_st.tile([128, 1024], F32, name="st", tag="st")
        nc.tensor.matmul(
            st[:, 0:512], kT[0:DH, m0:m1], qT[0:DH, n0:n1],
            start=True, stop=True, tile_position=(0, 0),
        )
        nc.tensor.matmul(
            st[:, 512:1024], kT[DH:128, m0:m1], qT[DH:128, n0:n1],
            start=True, stop=True, tile_position=(64, 0),
        )
        pt = ppool.tile([128, 1024], BF16, name="pt", tag="pt")
        nc.scalar.activation(out=pt, in_=st, func=EXP, bias=0.0, scale=SCALE)
        return pt

    def pv(mc, pt, oaug):
        first, last = mc == 0, mc == NM - 1
        for h in range(2):
            nc.tensor.matmul(
                oaug[h], Vall[:, h, mc, 0 : DH + 1],
                pt[:, h * 512 : (h + 1) * 512],
                start=first, stop=last,
            )

    def attn_mc(s, mc, oaug):
        pv(mc, scores_exp(s, mc), oaug)

    def mk_oaug():
        return [
            ps_oaug.tile([DH + 1, 512], F32, name=f"oaug{h}", tag="oaug")
            for h in range(2)
        ]

    def fin_pre(s, oaug, last=False):
        """Evacuate oaug, extract denominators, compute reciprocals.

        The [1, 1024] denominator row (row 64 of oaug) is repartitioned to
        [128, 8] via an SBUF->SBUF DMA so the iterative-divide reciprocal
        runs on all DVE lanes, then linearized back to a single-partition
        [1, 1024] row for the PE broadcast in fin_post.
        """
        oaug_sb = fpool.tile([DH + 1, 1024], F32, name="oaug_sb", tag="oaug_sb")
        if last:
            # tail: pull the den row via the (idle) ScalarE concurrently
            # with the DVE evacuation so the recip chain starts sooner
            den_sb = fpool.tile([1, 1024], F32, name="den_sb", tag="den_sb")
            for h in range(2):
                nc.scalar.copy(
                    out=den_sb[:, h * 512 : (h + 1) * 512],
                    in_=oaug[h][DH : DH + 1, :],
                )
                nc.vector.tensor_copy(
                    out=oaug_sb[0:DH, h * 512 : (h + 1) * 512],
                    in_=oaug[h][0:DH, :],
                )
            den_src = den_sb
        else:
            for h in range(2):
                nc.vector.tensor_copy(
                    out=oaug_sb[:, h * 512 : (h + 1) * 512], in_=oaug[h]
                )
            den_src = oaug_sb[DH : DH + 1, :]
        den_p = fpool.tile([128, 8], F32, name="den_p", tag="den_p")
        nc.sync.dma_start(out=den_p, in_=den_src)
        rec_p = fpool.tile([128, 8], BF16, name="rec_p", tag="rec_p")
        with nc.allow_low_precision(reason="softmax denom reciprocal at bf16"):
            nc.vector.reciprocal(out=rec_p, in_=den_p)
        rec_row = fpool.tile([1, 1024], BF16, name="rec_row", tag="rec_row")
        nc.sync.dma_start(out=rec_row, in_=rec_p)
        return oaug_sb, rec_row

    def fin_post(s, oaug_sb, rec_row):
        """OT[:, s-slice] = oaug[0:64] * (1/den), denominators broadcast
        across partitions with a K=1 PE matmul (rep = ones.T @ rec_row)."""
        n0, n1 = s * 512, (s + 1) * 512
        for h in range(2):
            rep = ps_acc.tile([DH, 512], F32, name="rep", tag="acc")
            nc.tensor.matmul(
                rep, ones1, rec_row[:, h * 512 : (h + 1) * 512],
                start=True, stop=True,
            )
            nc.vector.tensor_mul(
                out=OT[h * DH : (h + 1) * DH, n0:n1],
                in0=oaug_sb[0:DH, h * 512 : (h + 1) * 512],
                in1=rep,
            )

    def mk_osb():
        return fpool.tile([128, 4, 1024], BF16, name="osb", tag="osb")

    def outproj_nt(s, nt, osb):
        col = (s * 4 + nt) * 128
        for piece in range(2):
            c0, c1 = piece * 512, (piece + 1) * 512
            ops = ps_acc.tile([128, 512], F32, name="ops", tag="acc")
            nc.tensor.matmul(
                ops, OT[:, col : col + 128], wo_s[:, c0:c1],
                start=True, stop=True,
            )
            nc.vector.tensor_add(
                out=osb[:, nt, c0:c1], in0=ops, in1=bo_rep[:, c0:c1]
            )
        nc.sync.dma_start(
            out=out_d.ap()[:, (s * 4 + nt) * 1024 : (s * 4 + nt + 1) * 1024],
            in_=osb[:, nt, :],
        )

    qaccs = {}

    def qproj_part(s, k):
        """One k-chunk of the q projection (split so the scheduler can slot
        single matmuls into the ScalarE-paced slack without stalling the
        scores chain)."""
        if k == 0:
            qaccs[s] = ps_acc.tile([128, 512], F32, name="acc", tag="acc")
        acc = qaccs[s]
        nc.tensor.matmul(
            acc, wq_s[:, k, :], xs[:, s, k, :],
            start=(k == 0), stop=(k == NK - 1),
        )
        if k == NK - 1:
            nc.vector.tensor_copy(out=qT[:, s * 512 : (s + 1) * 512], in_=acc)

    def outproj_piece(s, nt, piece, osb):
        col = (s * 4 + nt) * 128
        c0, c1 = piece * 512, (piece + 1) * 512
        ops = ps_acc.tile([128, 512], F32, name="ops", tag="acc")
        nc.tensor.matmul(
            ops, OT[:, col : col + 128], wo_s[:, c0:c1], start=True, stop=True
        )
        nc.vector.tensor_add(out=osb[:, nt, c0:c1], in0=ops, in1=bo_rep[:, c0:c1])
        if piece == 1:
            nc.sync.dma_start(
                out=out_d.ap()[:, (s * 4 + nt) * 1024 : (s * 4 + nt + 1) * 1024],
                in_=osb[:, nt, :],
            )

    # ---- schedule ----
    # Emission order == scheduler priority.  One flat software-pipelined
    # loop over all 64 (s, mc) attention steps: the scores for step i+1 are
    # emitted before the PV of step i, so the ScalarE exp stream (the
    # kernel's pacer, ~71us) is one continuous priority band that crosses
    # n-chunk boundaries; projections / out-projection / v-transposes are
    # emitted after (lower priority) and fill the PE slack.
    oaug_cur = mk_oaug()
    osb_cur = None
    k_proj(0)
    qproj(0)
    pts = {0: scores_exp(0, 0)}
    pts[1] = scores_exp(0, 1)
    for i in range(NS * NM):
        s, mc = divmod(i, NM)
        ni = i + 2
        if ni < NS * NM:
            ns_, nmc = divmod(ni, NM)
            if ns_ == 0 and nmc % 4 == 0:
                k_proj(nmc // 4)
            pts[ni] = scores_exp(ns_, nmc)
        if s == 0:
            # v/transpose groups trail the k-chain by data arrival; PV lags
            # 3 steps so its slot grants come after the next k-projection's.
            if mc % 4 == 3 and mc < 15:
                g = mc // 4
                v_proj(g)
                v_trans(g)
            if mc == 8:
                qproj(1)
            if mc >= 3 and mc < 15:
                pv(mc - 3, pts.pop(i - 3), oaug_cur)
            if mc == 15:
                v_proj(3)
                v_trans(3)
                for j in range(12, 16):
                    pv(j, pts.pop(s * NM + j), oaug_cur)
        else:
            pv(mc, pts.pop(i), oaug_cur)
            if 4 <= mc <= 11 and s + 1 < NS:
                qproj_part(s + 1, mc - 4)
            if 6 <= mc <= 13:
                outproj_piece(s - 1, (mc - 6) // 2, (mc - 6) % 2, osb_cur)
        if s >= 1 and mc == 1:
            fin_post(s - 1, *fin_pending)
        if mc == NM - 1:
            fin_pending = fin_pre(s, oaug_cur, last=(s == NS - 1))
            if s + 1 < NS:
                oaug_cur = mk_oaug()
                osb_cur = mk_osb()

    fin_post(NS - 1, *fin_pending)
    osb_cur = mk_osb()
    for nt in range(4):
        for piece in range(2):
            outproj_piece(NS - 1, nt, piece, osb_cur)

    ctx.close()


_NC = None


def _get_nc():
    global _NC
    if _NC is None:
        _NC = build_nc()
    return _NC


def _bf16():
    import ml_dtypes

    return ml_dtypes.bfloat16


def _swizzle_w(w):
    """[1024, 128] -> [128, 8*128]: chunk k of the contraction dim lands in
    column block k, so the device DMA is fully contiguous."""
    return np.ascontiguousarray(
        np.asarray(w, np.float32).reshape(NK, 128, F).transpose(1, 0, 2)
        .reshape(128, NK * F).astype(_bf16())
    )


def _swizzle_act(a):
    """[n=2048, d=1024] -> [128, (s, k, j)] with [p, s*4096 + k*512 + j] =
    a[s*512 + j, k*128 + p]."""
    at = np.asarray(a, np.float32).T  # [1024, 2048]
    return np.ascontiguousarray(
        at.reshape(NK, 128, NS, 512).transpose(1, 2, 0, 3).reshape(128, NS * NK * 512)
        .astype(_bf16())
    )


def shard_inputs(x, context, Wq, Wk, Wv, Wo, bo):
    x = np.asarray(x, np.float32)
    context = np.asarray(context, np.float32)
    Wq = np.asarray(Wq, np.float32)
    Wk = np.asarray(Wk, np.float32)
    Wv = np.asarray(Wv, np.float32)
    Wo = np.asarray(Wo, np.float32)
    bo = np.asarray(bo, np.float32)

    bf = _bf16()
    x_sw = [_swizzle_act(x[b]) for b in range(x.shape[0])]
    c_sw = [_swizzle_act(context[b]) for b in range(context.shape[0])]
    zero_bo = np.zeros((1, D), bf)
    in_maps = []
    for c in range(8):
        b, hp = divmod(c, 4)
        f0 = hp * F
        in_maps.append(
            {
                "x_sw": x_sw[b],
                "c_sw": c_sw[b],
                "wq": _swizzle_w(Wq[:, f0 : f0 + F]),
                "wk": _swizzle_w(Wk[:, f0 : f0 + F]),
                "wv": _swizzle_w(Wv[:, f0 : f0 + F]),
                "wo": np.ascontiguousarray(Wo[f0 : f0 + F, :]).astype(bf),
                "bo": bo.reshape(1, D).astype(bf) if hp == 0 else zero_bo,
            }
        )
    return in_maps


def unswizzle_out(res):
    """[128, NS*4*1024] bf16 -> [2048, 1024] f32."""
    r = np.asarray(res, np.float32).reshape(128, NS, 4, 1024)
    return r.transpose(1, 2, 0, 3).reshape(SEQ, D)


def kernel(x, context, Wq, Wk, Wv, Wo, bo):
    from concourse.bass_utils import run_bass_kernel_spmd

    in_maps = shard_inputs(x, context, Wq, Wk, Wv, Wo, bo)
    nc = _get_nc()
    res = run_bass_kernel_spmd(nc, in_maps, list(range(8)))
    out = np.zeros((2, SEQ, D), np.float32)
    for c in range(8):
        out[c // 4] += unswizzle_out(res.results[c]["out_sw"])
    return out
